# revision 56
# baseline (speedup 1.0000x reference)
"""DenseCRF loss kernel for Trainium2, data-parallel over batch on 8 NeuronCores.

reference:
  seg = bilinear_resize(segmentations, 128->64)            # [N,K,64,64]
  f_i = [x_i/50, y_i/50, r_i/15, g_i/15, b_i/15]           # 5-dim bilateral feature
  W_ij = exp(-0.5*|f_i - f_j|^2)                           # [P,P], P=4096
  loss = WEIGHT * (-sum_k s_k^T W s_k) / N

Per core (1 image): W block = exp(G - q_i - q_j) with G the 5-d Gram matrix.
G is computed on the TensorEngine as a 22-row bf16 matmul where every feature is
split hi/lo into two bf16 values (bf16 products are exact in the fp32 PSUM
accumulator). -q_j rides two bf16 aux rows of FB; -q_i is the fp32 per-partition
bias of the Exp. The exp'd block (bf16) is contracted against the resized
segmentation with PSUM accumulation; a fused DVE multiply+reduce forms the
scalar, host sums 8 cores.

W is symmetric: only diagonal + strictly-upper 512x512 units are computed.
Strictly-upper blocks get a factor 2, folded either into a second exp bias
qcol2 = qcol + ln2 (both halves of a 1024-wide block upper) or into a doubled
seg-transpose STt2 (only the right half upper).

Full-width (1024) blocks exp on the ACT engine; the 16 diagonal half-blocks
exp on Pool/DVE via an exp2 bit trick (round via +1.5*2^23 magic, 2^i built by
shifting the mantissa bits, quadratic 2^f poly), keeping ACT the only hot
engine for exp while Pool/DVE absorb the rest.

Pass (column-group-pair) order is 2,0,1,3: pass 2's 20 full blocks keep ACT
busy while the PE runs the FA/FB assembly and segmentation-resize pipeline;
pass 2's contraction matmuls are dripped into later passes' slack.
"""

import sys

sys.path.insert(0, "/opt/trn_rl_repo")

import numpy as np
import ml_dtypes

import concourse.bass as bass
import concourse.tile as tile
from concourse import bacc, bass_isa, mybir
from concourse.bass_utils import run_bass_kernel_spmd

F32 = mybir.dt.float32
F32R = mybir.dt.float32r
I32 = mybir.dt.int32
BF16 = mybir.dt.bfloat16
AF = mybir.ActivationFunctionType
ALU = mybir.AluOpType
BF = ml_dtypes.bfloat16

N, C, K = 8, 3, 21
H, W = 64, 64
P = H * W  # 4096
SIGMA_RGB = 15.0
SXY = 100.0 * 0.5  # sigma_xy * scale
WEIGHT = 1e-8
NB = 32  # 128-row chunks of P
NF = 22  # feature rows of the Gram contraction
LN2 = float(np.log(2.0))
LOG2E = float(np.log2(np.e))
MAGIC = 12582912.0  # 1.5 * 2^23
SHIFTK = float(MAGIC - 127.0 - 8388608.0)  # t - SHIFTK = 2^23 + (i + 127)


def _exp2_poly():
    """Least-squares fit of 2^f on [-0.5, 0.5] with relative weighting."""
    f = np.linspace(-0.5, 0.5, 20001)
    w = 1.0 / np.exp2(f)
    A = np.stack([np.ones_like(f), f, f * f], 1) * w[:, None]
    c, *_ = np.linalg.lstsq(A, np.ones_like(f), rcond=None)
    return [float(v) for v in c]


E2C0, E2C1, E2C2 = _exp2_poly()


def _resize_matrix():
    """[64,128] weights of jax.image.resize(..., method='bilinear') along one dim
    (triangle kernel, antialias=True, scale=0.5, renormalized)."""
    y = np.arange(128, dtype=np.float64)[:, None]
    sample = 2.0 * np.arange(64, dtype=np.float64)[None, :] + 0.5
    w = np.maximum(0.0, 1.0 - 0.5 * np.abs(y - sample))
    w = w / w.sum(axis=0, keepdims=True)
    return np.ascontiguousarray(w.T.astype(np.float32))  # [64,128]


def _consts():
    R = _resize_matrix()  # [64,128]
    rtf = np.ascontiguousarray(R.T)  # [128,64] f32
    rtb = rtf.astype(BF)
    idf = np.eye(128, dtype=np.float32)
    idb = idf.astype(BF)
    i = np.arange(P, dtype=np.float32)
    px = (i % 64).astype(np.float32) / np.float32(SXY)
    py = (i // 64).astype(np.float32) / np.float32(SXY)
    pos = np.stack([px, py])  # [2,P] f32
    ph2 = pos.astype(BF)
    pl2 = (pos - ph2.astype(np.float32)).astype(BF)
    pf2 = ph2.astype(np.float32) + pl2.astype(np.float32)  # exact f~ for positions
    # feature-major skeletons [r=128, (b, f)] with the constant slots filled:
    # position hi/lo replicas (+ ones rows for FA); color/q slots zero.
    phr = ph2.reshape(2, NB, 128)  # [2, b, r]
    plr = pl2.reshape(2, NB, 128)
    fcbA = np.zeros((128, NB, NF), dtype=BF)
    fcbB = np.zeros((128, NB, NF), dtype=BF)
    for d in range(2):
        hi = phr[d].T  # [r, b]
        lo = plr[d].T
        fcbA[:, :, 0 + d] = hi
        fcbA[:, :, 5 + d] = hi
        fcbA[:, :, 10 + d] = lo
        fcbA[:, :, 15 + d] = lo
        fcbB[:, :, 0 + d] = hi
        fcbB[:, :, 5 + d] = lo
        fcbB[:, :, 10 + d] = hi
        fcbB[:, :, 15 + d] = lo
    fcbA[:, :, 20:22] = np.ones((128, NB, 2), dtype=BF)
    fcbA = np.ascontiguousarray(fcbA.reshape(128, NB * NF))
    fcbB = np.ascontiguousarray(fcbB.reshape(128, NB * NF))
    # -0.5*(px^2+py^2) in [r, b] layout
    nqpos = (-0.5 * (pf2[0] ** 2 + pf2[1] ** 2)).reshape(NB, 128).T
    nqpos = np.ascontiguousarray(nqpos.astype(np.float32))
    return dict(rtf=rtf, rtb=rtb, idf=idf, idb=idb, fcbA=fcbA, fcbB=fcbB,
                nqpos=nqpos)


def _build():
    nc = bacc.Bacc()
    images_d = nc.dram_tensor("images", [C, H, W], F32, kind="ExternalInput")
    seg_d = nc.dram_tensor("segmentations", [K, 128, 128], F32, kind="ExternalInput")
    rtf_d = nc.dram_tensor("rtf", [128, 64], F32, kind="ExternalInput")
    rtb_d = nc.dram_tensor("rtb", [128, 64], BF16, kind="ExternalInput")
    idf_d = nc.dram_tensor("idf", [128, 128], F32, kind="ExternalInput")
    idb_d = nc.dram_tensor("idb", [128, 128], BF16, kind="ExternalInput")
    fcbA_d = nc.dram_tensor("fcbA", [128, NB * NF], BF16, kind="ExternalInput")
    fcbB_d = nc.dram_tensor("fcbB", [128, NB * NF], BF16, kind="ExternalInput")
    nqpos_d = nc.dram_tensor("nqpos", [128, NB], F32, kind="ExternalInput")
    out_d = nc.dram_tensor("out", [K, 1], F32, kind="ExternalOutput")

    inv15 = float(np.float32(1.0) / np.float32(SIGMA_RGB))

    with tile.TileContext(nc) as tc:
        with tc.tile_pool(name="persist", bufs=1) as pp:
            FA = pp.tile([NF, P], BF16, tag="FA")
            FB = pp.tile([NF, P], BF16, tag="FB")
            img_cb = pp.tile([3 * NB, 128], F32, tag="img_cb")
            img_rc = pp.tile([128, 3 * NB], F32, tag="img_rc")
            fh = pp.tile([128, 3 * NB], BF16, tag="fh")
            fl = pp.tile([128, 3 * NB], BF16, tag="fl")
            fsq = pp.tile([128, 3 * NB], F32, tag="fsq")
            csum = pp.tile([128, NB], F32, tag="csum")
            qcol = pp.tile([128, NB], F32, tag="qcol")  # = -q (exp bias)
            qcol2 = pp.tile([128, NB], F32, tag="qcol2")
            nqh = pp.tile([128, NB], BF16, tag="nqh")
            nql = pp.tile([128, NB], BF16, tag="nql")
            nqpos_s = pp.tile([128, NB], F32, tag="nqpos")
            fcbA_s = pp.tile([128, NB * NF], BF16, tag="fcbA")
            fcbB_s = pp.tile([128, NB * NF], BF16, tag="fcbB")
            rtf_s = pp.tile([128, 64], F32, tag="rtf")
            rtb_s = pp.tile([128, 64], BF16, tag="rtb")
            idf_s = pp.tile([128, 128], F32, tag="idf")
            idb_s = pp.tile([128, 128], BF16, tag="idb")
            seg_s = pp.tile([128, K * 128], F32, tag="seg")
            A_sb = pp.tile([64, K * 128], BF16, tag="A_sb")
            At = pp.tile([128, K * 64], BF16, tag="At")
            Srow = pp.tile([K, P], F32, tag="Srow")
            STt = pp.tile([128, NB * K], BF16, tag="STt")
            STt2 = pp.tile([128, NB * K], BF16, tag="STt2")
            partials = pp.tile([K, 8], F32, tag="partials")
            pr1 = pp.tile([K, 1], F32, tag="pr1")

            # ---- DMAs ----
            # ACT/HWDGE: idf only (needed by the img transpose; ACT queue must
            # stay free for exps).
            nc.scalar.dma_start(idf_s[:], idf_d[:])
            # SP/HWDGE: img first (heads the q-chain; HWDGE issue beats
            # SWDGE by ~1us), then everything else in usage order; the big
            # seg transfer last so the small consts win the DMA engines
            # first (it's only needed by the resize, ~15us in).
            segr = seg_d.rearrange("k y x -> y k x")
            nc.sync.dma_start(img_cb[:], images_d.rearrange(
                "c (b h2) w -> (c b) (h2 w)", h2=2))
            nc.sync.dma_start(fcbA_s[:], fcbA_d[:])
            nc.sync.dma_start(fcbB_s[:], fcbB_d[:])
            nc.sync.dma_start(nqpos_s[:], nqpos_d[:])
            nc.sync.dma_start(idb_s[:], idb_d[:])
            nc.sync.dma_start(rtf_s[:], rtf_d[:])
            nc.sync.dma_start(rtb_s[:], rtb_d[:])
            nc.sync.dma_start(seg_s[:, 0:12 * 128], segr[:, 0:12, :])
            nc.sync.dma_start(seg_s[:, 12 * 128:], segr[:, 12:, :])

            with (
                tc.tile_pool(name="gps", bufs=2, space="PSUM") as gps,
                tc.tile_pool(name="acps", bufs=1, space="PSUM") as acps,
                tc.tile_pool(name="ep", bufs=26) as ep,
                tc.tile_pool(name="xp", bufs=2) as xp,
                tc.tile_pool(name="scp", bufs=2) as scp,
            ):
                # preload the ACT function table before inputs arrive so the
                # first real exp doesn't pay the ~1.3us table load
                warm = xp.tile([1, 1], F32, tag="warm", name="warm")
                nc.gpsimd.memset(warm[:], 0.0)
                nc.scalar.activation(warm[:], warm[:], AF.Exp)

                # ---- q chain (position-partition layouts; ACT-free) ----
                ips = gps.tile([128, 3 * NB], F32, tag="ps", name="ips")
                nc.tensor.transpose(ips[:], img_cb[:], idf_s[0:3 * NB, 0:3 * NB])
                nc.gpsimd.tensor_copy(img_rc[:], ips[:])
                # fh = bf16(img/15); fl = img/15 - fh; fsq = (img/15)^2
                nc.gpsimd.tensor_scalar_mul(fh[:], img_rc[:], inv15)
                nc.vector.scalar_tensor_tensor(
                    fsq[:], img_rc[:], inv15 * inv15, img_rc[:],
                    ALU.mult, ALU.mult)
                nc.vector.scalar_tensor_tensor(
                    fl[:], img_rc[:], inv15, fh[:], ALU.mult, ALU.subtract)
                nc.vector.tensor_add(csum[:], fsq[:, 0:NB], fsq[:, NB:2 * NB])
                nc.vector.tensor_add(csum[:], csum[:], fsq[:, 2 * NB:3 * NB])
                nc.vector.scalar_tensor_tensor(
                    qcol[:], csum[:], -0.5, nqpos_s[:], ALU.mult, ALU.add)
                nc.gpsimd.tensor_copy(nqh[:], qcol[:])  # bf16 rounding
                nc.vector.tensor_sub(nql[:], qcol[:], nqh[:])
                nc.gpsimd.tensor_scalar_add(qcol2[:], qcol[:], LN2)

                # ---- assemble feature-major tiles (color + q slots) ----
                a3 = fcbA_s[:].rearrange("r (b f) -> r b f", f=NF)
                b3 = fcbB_s[:].rearrange("r (b f) -> r b f", f=NF)
                fh3 = fh[:].rearrange("r (c b) -> r b c", c=3)
                fl3 = fl[:].rearrange("r (c b) -> r b c", c=3)
                # a3 colors on Pool while DVE finishes the q chain; b3 colors
                # early too, its nq slots last (they gate the FB transposes)
                nc.gpsimd.tensor_copy(a3[:, :, 2:5], fh3)
                nc.gpsimd.tensor_copy(b3[:, :, 2:5], fh3)
                nc.gpsimd.tensor_copy(a3[:, :, 7:10], fh3)
                nc.gpsimd.tensor_copy(b3[:, :, 12:15], fh3)
                nc.gpsimd.tensor_copy(a3[:, :, 12:15], fl3)
                nc.gpsimd.tensor_copy(b3[:, :, 7:10], fl3)
                nc.gpsimd.tensor_copy(a3[:, :, 17:20], fl3)
                nc.gpsimd.tensor_copy(b3[:, :, 17:20], fl3)
                nc.gpsimd.tensor_copy(b3[:, :, 20:21], nqh[:].unsqueeze(2))
                nc.gpsimd.tensor_copy(b3[:, :, 21:22], nql[:].unsqueeze(2))

                # ---- FA/FB columns via per-chunk PE transposes ----
                def emit_fab(which, b0):
                    src, dst = (a3, FA) if which == "A" else (b3, FB)
                    eng = nc.gpsimd if which == "A" else nc.vector
                    fps = gps.tile([NF, 8 * 128], BF16, tag="ps",
                                   name=f"f{which}{b0}")
                    for b in range(b0, b0 + 8):
                        nc.tensor.transpose(
                            fps[:, (b - b0) * 128:(b - b0 + 1) * 128],
                            src[:, b, :], idb_s[:, 0:128])
                    eng.tensor_copy(dst[:, b0 * 128:(b0 + 8) * 128], fps[:])

                # ---- helpers ----
                def pass_chunks(p):
                    ch = []
                    for b in range(8 * p):
                        ch.append((b, 2, qcol2, [(0, STt), (1, STt)]))
                    for b in range(8 * p, 8 * p + 4):
                        ch.append((b, 2, qcol, [(0, STt), (1, STt2)]))
                    for b in range(8 * p + 4, 8 * p + 8):
                        ch.append((b, 1, qcol, [(1, STt)]))
                    return ch

                def split_chunks(p):
                    ch = pass_chunks(p)
                    return ([c for c in ch if c[1] == 2],
                            [c for c in ch if c[1] == 1])

                half_ctr = [0]

                def emit_exp2(eng, et, gt, bias_ap, w):
                    """exp(gt + bias) via 2^y bit trick on Pool or DVE."""
                    n = half_ctr[0]
                    sl = slice(0, 512 * w)
                    y = xp.tile([128, 512], F32, tag="y", name=f"y{n}")
                    t = xp.tile([128, 512], F32, tag="t", name=f"t{n}")
                    fr = xp.tile([128, 512], F32, tag="fr", name=f"fr{n}")
                    s2 = xp.tile([128, 512], F32, tag="s2", name=f"s2{n}")
                    u = xp.tile([128, 512], I32, tag="u", name=f"u{n}")
                    h1 = xp.tile([128, 512], F32, tag="h1", name=f"h1{n}")
                    h2 = xp.tile([128, 512], F32, tag="h2", name=f"h2{n}")
                    eng.tensor_scalar(y[:], gt[:, sl], bias_ap, LOG2E,
                                      ALU.add, ALU.mult)
                    eng.tensor_scalar(t[:], y[:], MAGIC, MAGIC - 126.0,
                                      ALU.add, ALU.max)
                    eng.scalar_tensor_tensor(fr[:], t[:], MAGIC, y[:],
                                             ALU.subtract, ALU.subtract)
                    eng.tensor_scalar(s2[:], t[:], SHIFTK, None, ALU.subtract)
                    eng.tensor_scalar(u[:], s2[:].bitcast(I32), 23, None,
                                      ALU.logical_shift_left)
                    eng.tensor_scalar(h1[:], fr[:], E2C2, -E2C1,
                                      ALU.mult, ALU.add)
                    eng.tensor_tensor(h2[:], h1[:], fr[:], ALU.mult)
                    eng.scalar_tensor_tensor(et[:, sl], h2[:], E2C0,
                                             u[:].bitcast(F32), ALU.add, ALU.mult)
                    half_ctr[0] += 1

                def emit_gram_exp(p, b, width, biast):
                    g0 = 2 * p
                    fa_b = FA[:, b * 128:(b + 1) * 128]
                    if width == 2:
                        gt = gps.tile([128, 1024], F32, tag="g", name=f"g{p}_{b}")
                        nc.tensor.matmul(
                            gt[:, 0:512], fa_b, FB[:, g0 * 512:(g0 + 1) * 512],
                            start=True, stop=True)
                        nc.tensor.matmul(
                            gt[:, 512:1024], fa_b,
                            FB[:, (g0 + 1) * 512:(g0 + 2) * 512],
                            start=True, stop=True)
                    else:
                        # half blocks get their own psum ring so the ACT gram
                        # ring never waits on a slow Pool/DVE exp chain
                        gt = gps.tile([128, 512], F32, tag="ps", name=f"g{p}_{b}")
                        nc.tensor.matmul(
                            gt[:, 0:512], fa_b,
                            FB[:, (g0 + 1) * 512:(g0 + 2) * 512],
                            start=True, stop=True)
                    et = ep.tile([128, 1024], BF16, tag="e", name=f"e{p}_{b}")
                    if width == 2:
                        nc.scalar.activation(
                            et[:], gt[:], AF.Exp, bias=biast[:, b:b + 1])
                    elif p == 0:
                        # final pass: ACT is idling into the tail and a slow
                        # Pool/DVE chain here would gate the last accs
                        nc.scalar.activation(
                            et[:, 0:512], gt[:, 0:512], AF.Exp,
                            bias=biast[:, b:b + 1])
                    else:
                        eng = nc.gpsimd if half_ctr[0] % 2 == 0 else nc.vector
                        emit_exp2(eng, et, gt, biast[:, b:b + 1], 1)
                    return et

                # one acc slab; four [21,512] pass-pairs packed by (partition
                # base in {0,32,64}) x (column half). P3 reuses P0's regions --
                # its accs all follow finals0 in program order.
                slab = acps.tile([128, 1024], F32, tag="acc", name="accslab")
                acc_pairs = {
                    2: [slab[0:21, 0:512], slab[32:53, 0:512]],
                    0: [slab[64:85, 0:512], slab[64:85, 512:1024]],
                    1: [slab[0:21, 512:1024], slab[32:53, 512:1024]],
                    3: [slab[64:85, 0:512], slab[64:85, 512:1024]],
                }

                PASS_ORDER = (2, 0, 1, 3)

                def emit_finals(p):
                    # in-place product write: the PSUM acc region is both input
                    # and output, so a later pass reusing the region gets a
                    # write-after-write dependency on this read+write (the
                    # region is shared between passes 0 and 3)
                    accs = acc_pairs[p]
                    for gl in range(2):
                        g = 2 * p + gl
                        nc.vector.tensor_tensor_reduce(
                            accs[gl][:], accs[gl][:],
                            Srow[:, g * 512:(g + 1) * 512],
                            1.0, 0.0, ALU.mult, ALU.add,
                            partials[:, g:g + 1])

                remaining = {p: sum(len(c[3]) for c in pass_chunks(p))
                             for p in PASS_ORDER}
                gleft = {}
                for p in PASS_ORDER:
                    for b, w, bia, tg in pass_chunks(p):
                        for gl, _ in tg:
                            g = 2 * p + gl
                            gleft[g] = gleft.get(g, 0) + 1
                gstarted = set()
                debt = []  # (ready_idx, p, b, w, gl, stt, et)
                cur = [0]
                drip_on = [False]

                def push_accs(p, b, w, tg, et):
                    lag = 8 if (w == 1 and p != 0) else 1
                    for gl, stt in tg:
                        debt.append((cur[0] + lag, p, b, w, gl, stt, et))

                def drip(n, force=False):
                    done = 0
                    i = 0
                    while i < len(debt) and done < n:
                        ready, p, b, w, gl, stt, et = debt[i]
                        if not force and ready > cur[0]:
                            i += 1
                            continue
                        debt.pop(i)
                        g = 2 * p + gl
                        sl = et[:, gl * 512:(gl + 1) * 512] if w == 2 \
                            else et[:, 0:512]
                        gleft[g] -= 1
                        nc.tensor.matmul(
                            acc_pairs[p][gl][:], stt[:, b * K:(b + 1) * K], sl,
                            start=(g not in gstarted),
                            stop=(gleft[g] == 0),
                            skip_group_check=True)
                        gstarted.add(g)
                        remaining[p] -= 1
                        if remaining[p] == 0:
                            emit_finals(p)
                        done += 1

                def emit_chunk(p, c):
                    b, w, bia, tg = c
                    et = emit_gram_exp(p, b, w, bia)
                    push_accs(p, b, w, tg, et)
                    cur[0] += 1
                    if drip_on[0]:
                        drip(6 if w == 1 else 3)

                # ---- emission schedule ----
                # pass-2 fulls start as soon as FA batch 0 / FB batch 2 land;
                # the resize is emitted after just 10 chunks (its inputs are
                # ready by then), so only ~22 accs are deferred and the
                # post-resize stream stays ACT/PE balanced.
                f0, h0c = split_chunks(0)
                f1, h1c = split_chunks(1)
                f2, h2c = split_chunks(2)
                f3, h3c = split_chunks(3)
                emit_fab("A", 0)
                emit_fab("B", 16)
                for c in f2[0:4]:
                    emit_chunk(2, c)
                emit_fab("A", 8)
                emit_fab("A", 16)

                # ---- resize pipeline as stages, interleaved with chunks so
                # the stage round-trip latency hides behind gram/exp work ----
                def resize_stages():
                    stages = []
                    for ci, c0 in enumerate(range(0, K * 128, 512)):
                        def s(ci=ci, c0=c0):
                            c1 = min(c0 + 512, K * 128)
                            aps = gps.tile([64, 512], F32, tag="ps",
                                           name=f"aps{c0}")
                            nc.tensor.matmul(
                                aps[:, :c1 - c0], rtf_s[:].bitcast(F32R),
                                seg_s[:, c0:c1].bitcast(F32R),
                                start=True, stop=True)
                            eng = nc.gpsimd if ci % 2 == 0 else nc.vector
                            eng.tensor_copy(A_sb[:, c0:c1], aps[:, :c1 - c0])
                        stages.append(s)
                    for ki, k0 in enumerate(range(0, K, 8)):
                        def s(ki=ki, k0=k0):
                            k1 = min(k0 + 8, K)
                            tps = gps.tile([128, 64 * 8], BF16, tag="ps",
                                           name=f"tps{k0}")
                            for k in range(k0, k1):
                                nc.tensor.transpose(
                                    tps[:, (k - k0) * 64:(k - k0 + 1) * 64],
                                    A_sb[0:64, k * 128:(k + 1) * 128],
                                    idb_s[0:64, 0:64])
                            eng = nc.gpsimd if ki % 2 == 0 else nc.vector
                            eng.tensor_copy(At[:, k0 * 64:k1 * 64],
                                            tps[:, :(k1 - k0) * 64])
                        stages.append(s)
                    at3 = At[:, :].rearrange("x (k y) -> x k y", k=K, y=64)
                    for yb in range(8):
                        def s(yb=yb):
                            sps = gps.tile([K, 512], F32, tag="ps",
                                           name=f"sps{yb}")
                            for yl in range(8):
                                yp = yb * 8 + yl
                                nc.tensor.matmul(
                                    sps[:, yl * 64:(yl + 1) * 64],
                                    at3[:, :, yp], rtb_s[:],
                                    start=True, stop=True)
                            eng = nc.gpsimd if yb % 2 == 0 else nc.vector
                            eng.tensor_copy(Srow[:, yb * 512:(yb + 1) * 512],
                                            sps[:])
                        stages.append(s)
                    for bi, b0 in enumerate(range(0, NB, 8)):
                        def s(bi=bi, b0=b0):
                            t2 = gps.tile([128, K * 8], F32, tag="ps",
                                          name=f"t2_{b0}")
                            for b in range(b0, b0 + 8):
                                nc.tensor.transpose(
                                    t2[:, (b - b0) * K:(b - b0 + 1) * K],
                                    Srow[:, b * 128:(b + 1) * 128],
                                    idf_s[0:K, 0:K])
                            eng = nc.gpsimd if bi % 2 == 0 else nc.vector
                            eng.tensor_copy(STt[:, b0 * K:(b0 + 8) * K], t2[:])
                        stages.append(s)
                    stages.append(lambda: nc.gpsimd.tensor_scalar_mul(
                        STt2[:], STt[:], 2.0))
                    return stages

                stages = resize_stages()
                for c in f2[4:16]:
                    emit_chunk(2, c)
                    for s in stages[:2]:
                        s()
                    stages = stages[2:]
                for s in stages:
                    s()

                # post-resize: drip on (the pre-resize debt drains gradually
                # under ACT's exp backlog), remaining chunks with halves
                # spread WITHIN each pass only: pass acc-lifetimes must stay
                # disjoint along each PSUM-region chain (p2->p1, p0->p3),
                # since a pass's start=True would wipe a still-open previous
                # accumulation in the shared region.
                drip_on[0] = True
                # remaining batches now: their Pool/DVE copies enter the
                # queues ahead of the exp chains
                emit_fab("B", 0)
                emit_fab("B", 8)
                emit_fab("A", 24)
                emit_fab("B", 24)

                def pass_seq(fulls, halves):
                    # halves first-ish: a pass's last acc must be early enough
                    # that the region-chain partner's start never overtakes it
                    stride = max(1, len(fulls) // len(halves))
                    out, fi = [], 0
                    for h in halves:
                        out += [h] + fulls[fi:fi + stride]
                        fi += stride
                    return out + fulls[fi:]

                passmap = {}
                for p, (fs, hs) in ((0, (f0, h0c)), (1, (f1, h1c)),
                                    (2, (f2, h2c)), (3, (f3, h3c))):
                    for c in fs + hs:
                        passmap[id(c)] = p

                def emit_seq(seq):
                    for c in seq:
                        emit_chunk(passmap[id(c)], c)

                emit_seq(pass_seq(f2[16:20], h2c))
                emit_seq(pass_seq(f3, h3c))
                emit_seq(pass_seq(f1, h1c))
                emit_seq(pass_seq(f0, h0c))
                drip(len(debt), force=True)

                # ---- loss tail: per-class sums to host (it sums K classes
                # x 8 cores and applies -WEIGHT/N) ----
                nc.vector.tensor_reduce(
                    pr1[:], partials[:], mybir.AxisListType.X, ALU.add)
                nc.sync.dma_start(out_d[:], pr1[:])

    nc.finalize()
    return nc


_CACHE = {}


def _get_nc():
    if "nc" not in _CACHE:
        _CACHE["nc"] = _build()
    return _CACHE["nc"]


def kernel(images: np.ndarray, segmentations: np.ndarray) -> np.ndarray:
    images = np.ascontiguousarray(np.asarray(images, dtype=np.float32))
    segmentations = np.ascontiguousarray(np.asarray(segmentations, dtype=np.float32))
    assert images.shape == (N, C, H, W) and segmentations.shape == (N, K, 128, 128)
    nc = _get_nc()
    consts = _consts()
    in_maps = [
        {"images": images[n], "segmentations": segmentations[n], **consts}
        for n in range(N)
    ]
    res = run_bass_kernel_spmd(nc, in_maps, list(range(N)))
    total = sum(float(np.asarray(res.results[n]["out"], dtype=np.float64).sum())
                for n in range(N))
    total *= -WEIGHT / N
    return np.array([total], dtype=np.float32)


if __name__ == "__main__":
    rng = np.random.RandomState(0)
    img = rng.rand(N, C, H, W).astype(np.float32) * 255.0
    seg = rng.rand(N, K, 128, 128).astype(np.float32)
    print(kernel(img, seg))


# revision 57
# speedup vs baseline: 1.0099x; 1.0099x over previous
"""DenseCRF loss kernel for Trainium2, data-parallel over batch on 8 NeuronCores.

reference:
  seg = bilinear_resize(segmentations, 128->64)            # [N,K,64,64]
  f_i = [x_i/50, y_i/50, r_i/15, g_i/15, b_i/15]           # 5-dim bilateral feature
  W_ij = exp(-0.5*|f_i - f_j|^2)                           # [P,P], P=4096
  loss = WEIGHT * (-sum_k s_k^T W s_k) / N

Per core (1 image): W block = exp(G - q_i - q_j) with G the 5-d Gram matrix.
G is computed on the TensorEngine as a 22-row bf16 matmul where every feature is
split hi/lo into two bf16 values (bf16 products are exact in the fp32 PSUM
accumulator). -q_j rides two bf16 aux rows of FB; -q_i is the fp32 per-partition
bias of the Exp. The exp'd block (bf16) is contracted against the resized
segmentation with PSUM accumulation; a fused DVE multiply+reduce forms the
scalar, host sums 8 cores.

W is symmetric: only diagonal + strictly-upper 512x512 units are computed.
Strictly-upper blocks get a factor 2, folded either into a second exp bias
qcol2 = qcol + ln2 (both halves of a 1024-wide block upper) or into a doubled
seg-transpose STt2 (only the right half upper).

Full-width (1024) blocks exp on the ACT engine; the 16 diagonal half-blocks
exp on Pool/DVE via an exp2 bit trick (round via +1.5*2^23 magic, 2^i built by
shifting the mantissa bits, quadratic 2^f poly), keeping ACT the only hot
engine for exp while Pool/DVE absorb the rest.

Pass (column-group-pair) order is 2,0,1,3: pass 2's 20 full blocks keep ACT
busy while the PE runs the FA/FB assembly and segmentation-resize pipeline;
pass 2's contraction matmuls are dripped into later passes' slack.
"""

import sys

sys.path.insert(0, "/opt/trn_rl_repo")

import numpy as np
import ml_dtypes

import concourse.bass as bass
import concourse.tile as tile
from concourse import bacc, bass_isa, mybir
from concourse.bass_utils import run_bass_kernel_spmd

F32 = mybir.dt.float32
F32R = mybir.dt.float32r
I32 = mybir.dt.int32
BF16 = mybir.dt.bfloat16
AF = mybir.ActivationFunctionType
ALU = mybir.AluOpType
BF = ml_dtypes.bfloat16

N, C, K = 8, 3, 21
H, W = 64, 64
P = H * W  # 4096
SIGMA_RGB = 15.0
SXY = 100.0 * 0.5  # sigma_xy * scale
WEIGHT = 1e-8
NB = 32  # 128-row chunks of P
NF = 22  # feature rows of the Gram contraction
LN2 = float(np.log(2.0))
LOG2E = float(np.log2(np.e))
MAGIC = 12582912.0  # 1.5 * 2^23
SHIFTK = float(MAGIC - 127.0 - 8388608.0)  # t - SHIFTK = 2^23 + (i + 127)


def _exp2_poly():
    """Least-squares fit of 2^f on [-0.5, 0.5] with relative weighting."""
    f = np.linspace(-0.5, 0.5, 20001)
    w = 1.0 / np.exp2(f)
    A = np.stack([np.ones_like(f), f, f * f], 1) * w[:, None]
    c, *_ = np.linalg.lstsq(A, np.ones_like(f), rcond=None)
    return [float(v) for v in c]


E2C0, E2C1, E2C2 = _exp2_poly()


def _resize_matrix():
    """[64,128] weights of jax.image.resize(..., method='bilinear') along one dim
    (triangle kernel, antialias=True, scale=0.5, renormalized)."""
    y = np.arange(128, dtype=np.float64)[:, None]
    sample = 2.0 * np.arange(64, dtype=np.float64)[None, :] + 0.5
    w = np.maximum(0.0, 1.0 - 0.5 * np.abs(y - sample))
    w = w / w.sum(axis=0, keepdims=True)
    return np.ascontiguousarray(w.T.astype(np.float32))  # [64,128]


def _consts():
    R = _resize_matrix()  # [64,128]
    rtf = np.ascontiguousarray(R.T)  # [128,64] f32
    rtb = rtf.astype(BF)
    idf = np.eye(128, dtype=np.float32)
    idb = idf.astype(BF)
    i = np.arange(P, dtype=np.float32)
    px = (i % 64).astype(np.float32) / np.float32(SXY)
    py = (i // 64).astype(np.float32) / np.float32(SXY)
    pos = np.stack([px, py])  # [2,P] f32
    ph2 = pos.astype(BF)
    pl2 = (pos - ph2.astype(np.float32)).astype(BF)
    pf2 = ph2.astype(np.float32) + pl2.astype(np.float32)  # exact f~ for positions
    # feature-major skeletons [r=128, (b, f)] with the constant slots filled:
    # position hi/lo replicas (+ ones rows for FA); color/q slots zero.
    phr = ph2.reshape(2, NB, 128)  # [2, b, r]
    plr = pl2.reshape(2, NB, 128)
    fcbA = np.zeros((128, NB, NF), dtype=BF)
    fcbB = np.zeros((128, NB, NF), dtype=BF)
    for d in range(2):
        hi = phr[d].T  # [r, b]
        lo = plr[d].T
        fcbA[:, :, 0 + d] = hi
        fcbA[:, :, 5 + d] = hi
        fcbA[:, :, 10 + d] = lo
        fcbA[:, :, 15 + d] = lo
        fcbB[:, :, 0 + d] = hi
        fcbB[:, :, 5 + d] = lo
        fcbB[:, :, 10 + d] = hi
        fcbB[:, :, 15 + d] = lo
    fcbA[:, :, 20:22] = np.ones((128, NB, 2), dtype=BF)
    fcbA = np.ascontiguousarray(fcbA.reshape(128, NB * NF))
    fcbB = np.ascontiguousarray(fcbB.reshape(128, NB * NF))
    # -0.5*(px^2+py^2) in [r, b] layout
    nqpos = (-0.5 * (pf2[0] ** 2 + pf2[1] ** 2)).reshape(NB, 128).T
    nqpos = np.ascontiguousarray(nqpos.astype(np.float32))
    return dict(rtf=rtf, rtb=rtb, idf=idf, idb=idb, fcbA=fcbA, fcbB=fcbB,
                nqpos=nqpos)


def _build():
    nc = bacc.Bacc()
    images_d = nc.dram_tensor("images", [C, H, W], F32, kind="ExternalInput")
    seg_d = nc.dram_tensor("segmentations", [K, 128, 128], F32, kind="ExternalInput")
    rtf_d = nc.dram_tensor("rtf", [128, 64], F32, kind="ExternalInput")
    rtb_d = nc.dram_tensor("rtb", [128, 64], BF16, kind="ExternalInput")
    idf_d = nc.dram_tensor("idf", [128, 128], F32, kind="ExternalInput")
    idb_d = nc.dram_tensor("idb", [128, 128], BF16, kind="ExternalInput")
    fcbA_d = nc.dram_tensor("fcbA", [128, NB * NF], BF16, kind="ExternalInput")
    fcbB_d = nc.dram_tensor("fcbB", [128, NB * NF], BF16, kind="ExternalInput")
    nqpos_d = nc.dram_tensor("nqpos", [128, NB], F32, kind="ExternalInput")
    out_d = nc.dram_tensor("out", [K, 1], F32, kind="ExternalOutput")

    inv15 = float(np.float32(1.0) / np.float32(SIGMA_RGB))

    with tile.TileContext(nc) as tc:
        with tc.tile_pool(name="persist", bufs=1) as pp:
            FA = pp.tile([NF, P], BF16, tag="FA")
            FB = pp.tile([NF, P], BF16, tag="FB")
            img_cb = pp.tile([3 * NB, 128], F32, tag="img_cb")
            img_rc = pp.tile([128, 3 * NB], F32, tag="img_rc")
            fh = pp.tile([128, 3 * NB], BF16, tag="fh")
            fl = pp.tile([128, 3 * NB], BF16, tag="fl")
            fsq = pp.tile([128, 3 * NB], F32, tag="fsq")
            csum = pp.tile([128, NB], F32, tag="csum")
            qcol = pp.tile([128, NB], F32, tag="qcol")  # = -q (exp bias)
            qcol2 = pp.tile([128, NB], F32, tag="qcol2")
            nqh = pp.tile([128, NB], BF16, tag="nqh")
            nql = pp.tile([128, NB], BF16, tag="nql")
            nqpos_s = pp.tile([128, NB], F32, tag="nqpos")
            fcbA_s = pp.tile([128, NB * NF], BF16, tag="fcbA")
            fcbB_s = pp.tile([128, NB * NF], BF16, tag="fcbB")
            rtf_s = pp.tile([128, 64], F32, tag="rtf")
            rtb_s = pp.tile([128, 64], BF16, tag="rtb")
            idf_s = pp.tile([128, 128], F32, tag="idf")
            idb_s = pp.tile([128, 128], BF16, tag="idb")
            seg_s = pp.tile([128, K * 128], F32, tag="seg")
            A_sb = pp.tile([64, K * 128], BF16, tag="A_sb")
            At = pp.tile([128, K * 64], BF16, tag="At")
            Srow = pp.tile([K, P], F32, tag="Srow")
            STt = pp.tile([128, NB * K], BF16, tag="STt")
            STt2 = pp.tile([128, NB * K], BF16, tag="STt2")
            partials = pp.tile([K, 8], F32, tag="partials")
            pr1 = pp.tile([K, 1], F32, tag="pr1")

            # ---- DMAs ----
            # ACT/HWDGE: idf only (needed by the img transpose; ACT queue must
            # stay free for exps).
            nc.scalar.dma_start(idf_s[:], idf_d[:])
            # SP/HWDGE: img first (heads the q-chain; HWDGE issue beats
            # SWDGE by ~1us), then everything else in usage order; the big
            # seg transfer last so the small consts win the DMA engines
            # first (it's only needed by the resize, ~15us in).
            segr = seg_d.rearrange("k y x -> y k x")
            nc.sync.dma_start(img_cb[:], images_d.rearrange(
                "c (b h2) w -> (c b) (h2 w)", h2=2))
            nc.sync.dma_start(fcbA_s[:], fcbA_d[:])
            nc.sync.dma_start(fcbB_s[:], fcbB_d[:])
            nc.sync.dma_start(nqpos_s[:], nqpos_d[:])
            nc.sync.dma_start(idb_s[:], idb_d[:])
            nc.sync.dma_start(rtf_s[:], rtf_d[:])
            nc.sync.dma_start(rtb_s[:], rtb_d[:])
            nc.sync.dma_start(seg_s[:, 0:12 * 128], segr[:, 0:12, :])
            nc.sync.dma_start(seg_s[:, 12 * 128:], segr[:, 12:, :])

            with (
                tc.tile_pool(name="gps", bufs=2, space="PSUM") as gps,
                tc.tile_pool(name="acps", bufs=1, space="PSUM") as acps,
                tc.tile_pool(name="ep", bufs=26) as ep,
                tc.tile_pool(name="xp", bufs=2) as xp,
                tc.tile_pool(name="scp", bufs=2) as scp,
            ):
                # preload the ACT function table before inputs arrive so the
                # first real exp doesn't pay the ~1.3us table load
                warm = xp.tile([1, 1], F32, tag="warm", name="warm")
                nc.gpsimd.memset(warm[:], 0.0)
                nc.scalar.activation(warm[:], warm[:], AF.Exp)

                # ---- q chain (position-partition layouts; ACT-free) ----
                ips = gps.tile([128, 3 * NB], F32, tag="ps", name="ips")
                nc.tensor.transpose(ips[:], img_cb[:], idf_s[0:3 * NB, 0:3 * NB])
                nc.gpsimd.tensor_copy(img_rc[:], ips[:])
                # fh = bf16(img/15); fl = img/15 - fh; fsq = (img/15)^2
                nc.gpsimd.tensor_scalar_mul(fh[:], img_rc[:], inv15)
                nc.vector.scalar_tensor_tensor(
                    fsq[:], img_rc[:], inv15 * inv15, img_rc[:],
                    ALU.mult, ALU.mult)
                nc.vector.scalar_tensor_tensor(
                    fl[:], img_rc[:], inv15, fh[:], ALU.mult, ALU.subtract)
                nc.vector.tensor_add(csum[:], fsq[:, 0:NB], fsq[:, NB:2 * NB])
                nc.vector.tensor_add(csum[:], csum[:], fsq[:, 2 * NB:3 * NB])
                nc.vector.scalar_tensor_tensor(
                    qcol[:], csum[:], -0.5, nqpos_s[:], ALU.mult, ALU.add)
                nc.gpsimd.tensor_copy(nqh[:], qcol[:])  # bf16 rounding
                nc.vector.tensor_sub(nql[:], qcol[:], nqh[:])
                nc.gpsimd.tensor_scalar_add(qcol2[:], qcol[:], LN2)

                # ---- assemble feature-major tiles (color + q slots) ----
                a3 = fcbA_s[:].rearrange("r (b f) -> r b f", f=NF)
                b3 = fcbB_s[:].rearrange("r (b f) -> r b f", f=NF)
                fh3 = fh[:].rearrange("r (c b) -> r b c", c=3)
                fl3 = fl[:].rearrange("r (c b) -> r b c", c=3)
                # a3 colors on Pool while DVE finishes the q chain; b3 colors
                # early too, its nq slots last (they gate the FB transposes)
                nc.gpsimd.tensor_copy(a3[:, :, 2:5], fh3)
                nc.gpsimd.tensor_copy(b3[:, :, 2:5], fh3)
                nc.gpsimd.tensor_copy(a3[:, :, 7:10], fh3)
                nc.gpsimd.tensor_copy(b3[:, :, 12:15], fh3)
                nc.gpsimd.tensor_copy(a3[:, :, 12:15], fl3)
                nc.gpsimd.tensor_copy(b3[:, :, 7:10], fl3)
                nc.gpsimd.tensor_copy(a3[:, :, 17:20], fl3)
                nc.gpsimd.tensor_copy(b3[:, :, 17:20], fl3)
                nc.gpsimd.tensor_copy(b3[:, :, 20:21], nqh[:].unsqueeze(2))
                nc.gpsimd.tensor_copy(b3[:, :, 21:22], nql[:].unsqueeze(2))

                # ---- FA/FB columns via per-chunk PE transposes ----
                def emit_fab(which, b0):
                    src, dst = (a3, FA) if which == "A" else (b3, FB)
                    eng = nc.gpsimd if which == "A" else nc.vector
                    fps = gps.tile([NF, 8 * 128], BF16, tag="ps",
                                   name=f"f{which}{b0}")
                    for b in range(b0, b0 + 8):
                        nc.tensor.transpose(
                            fps[:, (b - b0) * 128:(b - b0 + 1) * 128],
                            src[:, b, :], idb_s[:, 0:128])
                    eng.tensor_copy(dst[:, b0 * 128:(b0 + 8) * 128], fps[:])

                # ---- helpers ----
                def pass_chunks(p):
                    ch = []
                    for b in range(8 * p):
                        ch.append((b, 2, qcol2, [(0, STt), (1, STt)]))
                    for b in range(8 * p, 8 * p + 4):
                        ch.append((b, 2, qcol, [(0, STt), (1, STt2)]))
                    for b in range(8 * p + 4, 8 * p + 8):
                        ch.append((b, 1, qcol, [(1, STt)]))
                    return ch

                def split_chunks(p):
                    ch = pass_chunks(p)
                    return ([c for c in ch if c[1] == 2],
                            [c for c in ch if c[1] == 1])

                half_ctr = [0]

                def emit_exp2(eng, et, gt, bias_ap, w):
                    """exp(gt + bias) via 2^y bit trick on Pool or DVE."""
                    n = half_ctr[0]
                    sl = slice(0, 512 * w)
                    y = xp.tile([128, 512], F32, tag="y", name=f"y{n}")
                    t = xp.tile([128, 512], F32, tag="t", name=f"t{n}")
                    fr = xp.tile([128, 512], F32, tag="fr", name=f"fr{n}")
                    s2 = xp.tile([128, 512], F32, tag="s2", name=f"s2{n}")
                    u = xp.tile([128, 512], I32, tag="u", name=f"u{n}")
                    h1 = xp.tile([128, 512], F32, tag="h1", name=f"h1{n}")
                    h2 = xp.tile([128, 512], F32, tag="h2", name=f"h2{n}")
                    eng.tensor_scalar(y[:], gt[:, sl], bias_ap, LOG2E,
                                      ALU.add, ALU.mult)
                    eng.tensor_scalar(t[:], y[:], MAGIC, MAGIC - 126.0,
                                      ALU.add, ALU.max)
                    eng.scalar_tensor_tensor(fr[:], t[:], MAGIC, y[:],
                                             ALU.subtract, ALU.subtract)
                    eng.tensor_scalar(s2[:], t[:], SHIFTK, None, ALU.subtract)
                    eng.tensor_scalar(u[:], s2[:].bitcast(I32), 23, None,
                                      ALU.logical_shift_left)
                    eng.tensor_scalar(h1[:], fr[:], E2C2, -E2C1,
                                      ALU.mult, ALU.add)
                    eng.tensor_tensor(h2[:], h1[:], fr[:], ALU.mult)
                    eng.scalar_tensor_tensor(et[:, sl], h2[:], E2C0,
                                             u[:].bitcast(F32), ALU.add, ALU.mult)
                    half_ctr[0] += 1

                def emit_gram_exp(p, b, width, biast):
                    g0 = 2 * p
                    fa_b = FA[:, b * 128:(b + 1) * 128]
                    if width == 2:
                        gt = gps.tile([128, 1024], F32, tag="g", name=f"g{p}_{b}")
                        nc.tensor.matmul(
                            gt[:, 0:512], fa_b, FB[:, g0 * 512:(g0 + 1) * 512],
                            start=True, stop=True)
                        nc.tensor.matmul(
                            gt[:, 512:1024], fa_b,
                            FB[:, (g0 + 1) * 512:(g0 + 2) * 512],
                            start=True, stop=True)
                    else:
                        # half blocks get their own psum ring so the ACT gram
                        # ring never waits on a slow Pool/DVE exp chain
                        gt = gps.tile([128, 512], F32, tag="ps", name=f"g{p}_{b}")
                        nc.tensor.matmul(
                            gt[:, 0:512], fa_b,
                            FB[:, (g0 + 1) * 512:(g0 + 2) * 512],
                            start=True, stop=True)
                    et = ep.tile([128, 1024], BF16, tag="e", name=f"e{p}_{b}")
                    if width == 2:
                        nc.scalar.activation(
                            et[:], gt[:], AF.Exp, bias=biast[:, b:b + 1])
                    else:
                        eng = nc.gpsimd if half_ctr[0] % 2 == 0 else nc.vector
                        emit_exp2(eng, et, gt, biast[:, b:b + 1], 1)
                    return et

                # one acc slab; four [21,512] pass-pairs packed by (partition
                # base in {0,32,64}) x (column half). P3 reuses P0's regions --
                # its accs all follow finals0 in program order.
                slab = acps.tile([128, 1024], F32, tag="acc", name="accslab")
                acc_pairs = {
                    2: [slab[0:21, 0:512], slab[32:53, 0:512]],
                    0: [slab[64:85, 0:512], slab[64:85, 512:1024]],
                    1: [slab[0:21, 512:1024], slab[32:53, 512:1024]],
                    3: [slab[64:85, 0:512], slab[64:85, 512:1024]],
                }

                PASS_ORDER = (2, 0, 1, 3)

                def emit_finals(p):
                    # in-place product write: the PSUM acc region is both input
                    # and output, so a later pass reusing the region gets a
                    # write-after-write dependency on this read+write (the
                    # region is shared between passes 0 and 3)
                    accs = acc_pairs[p]
                    for gl in range(2):
                        g = 2 * p + gl
                        nc.vector.tensor_tensor_reduce(
                            accs[gl][:], accs[gl][:],
                            Srow[:, g * 512:(g + 1) * 512],
                            1.0, 0.0, ALU.mult, ALU.add,
                            partials[:, g:g + 1])

                remaining = {p: sum(len(c[3]) for c in pass_chunks(p))
                             for p in PASS_ORDER}
                gleft = {}
                for p in PASS_ORDER:
                    for b, w, bia, tg in pass_chunks(p):
                        for gl, _ in tg:
                            g = 2 * p + gl
                            gleft[g] = gleft.get(g, 0) + 1
                gstarted = set()
                debt = []  # (ready_idx, p, b, w, gl, stt, et)
                cur = [0]
                drip_on = [False]

                def push_accs(p, b, w, tg, et):
                    lag = 8 if w == 1 else 1
                    for gl, stt in tg:
                        debt.append((cur[0] + lag, p, b, w, gl, stt, et))

                def drip(n, force=False):
                    done = 0
                    i = 0
                    while i < len(debt) and done < n:
                        ready, p, b, w, gl, stt, et = debt[i]
                        if not force and ready > cur[0]:
                            i += 1
                            continue
                        debt.pop(i)
                        g = 2 * p + gl
                        sl = et[:, gl * 512:(gl + 1) * 512] if w == 2 \
                            else et[:, 0:512]
                        gleft[g] -= 1
                        nc.tensor.matmul(
                            acc_pairs[p][gl][:], stt[:, b * K:(b + 1) * K], sl,
                            start=(g not in gstarted),
                            stop=(gleft[g] == 0),
                            skip_group_check=True)
                        gstarted.add(g)
                        remaining[p] -= 1
                        if remaining[p] == 0:
                            emit_finals(p)
                        done += 1

                def emit_chunk(p, c):
                    b, w, bia, tg = c
                    et = emit_gram_exp(p, b, w, bia)
                    push_accs(p, b, w, tg, et)
                    cur[0] += 1
                    if drip_on[0]:
                        drip(6 if w == 1 else 3)

                # ---- emission schedule ----
                # pass-2 fulls start as soon as FA batch 0 / FB batch 2 land;
                # the resize is emitted after just 10 chunks (its inputs are
                # ready by then), so only ~22 accs are deferred and the
                # post-resize stream stays ACT/PE balanced.
                f0, h0c = split_chunks(0)
                f1, h1c = split_chunks(1)
                f2, h2c = split_chunks(2)
                f3, h3c = split_chunks(3)
                emit_fab("A", 0)
                emit_fab("B", 16)
                for c in f2[0:4]:
                    emit_chunk(2, c)
                emit_fab("A", 8)
                emit_fab("A", 16)

                # ---- resize pipeline as stages, interleaved with chunks so
                # the stage round-trip latency hides behind gram/exp work ----
                def resize_stages():
                    stages = []
                    for ci, c0 in enumerate(range(0, K * 128, 512)):
                        def s(ci=ci, c0=c0):
                            c1 = min(c0 + 512, K * 128)
                            aps = gps.tile([64, 512], F32, tag="ps",
                                           name=f"aps{c0}")
                            nc.tensor.matmul(
                                aps[:, :c1 - c0], rtf_s[:].bitcast(F32R),
                                seg_s[:, c0:c1].bitcast(F32R),
                                start=True, stop=True)
                            eng = nc.gpsimd if ci % 2 == 0 else nc.vector
                            eng.tensor_copy(A_sb[:, c0:c1], aps[:, :c1 - c0])
                        stages.append(s)
                    for ki, k0 in enumerate(range(0, K, 8)):
                        def s(ki=ki, k0=k0):
                            k1 = min(k0 + 8, K)
                            tps = gps.tile([128, 64 * 8], BF16, tag="ps",
                                           name=f"tps{k0}")
                            for k in range(k0, k1):
                                nc.tensor.transpose(
                                    tps[:, (k - k0) * 64:(k - k0 + 1) * 64],
                                    A_sb[0:64, k * 128:(k + 1) * 128],
                                    idb_s[0:64, 0:64])
                            eng = nc.gpsimd if ki % 2 == 0 else nc.vector
                            eng.tensor_copy(At[:, k0 * 64:k1 * 64],
                                            tps[:, :(k1 - k0) * 64])
                        stages.append(s)
                    at3 = At[:, :].rearrange("x (k y) -> x k y", k=K, y=64)
                    for yb in range(8):
                        def s(yb=yb):
                            sps = gps.tile([K, 512], F32, tag="ps",
                                           name=f"sps{yb}")
                            for yl in range(8):
                                yp = yb * 8 + yl
                                nc.tensor.matmul(
                                    sps[:, yl * 64:(yl + 1) * 64],
                                    at3[:, :, yp], rtb_s[:],
                                    start=True, stop=True)
                            eng = nc.gpsimd if yb % 2 == 0 else nc.vector
                            eng.tensor_copy(Srow[:, yb * 512:(yb + 1) * 512],
                                            sps[:])
                        stages.append(s)
                    for bi, b0 in enumerate(range(0, NB, 8)):
                        def s(bi=bi, b0=b0):
                            t2 = gps.tile([128, K * 8], F32, tag="ps",
                                          name=f"t2_{b0}")
                            for b in range(b0, b0 + 8):
                                nc.tensor.transpose(
                                    t2[:, (b - b0) * K:(b - b0 + 1) * K],
                                    Srow[:, b * 128:(b + 1) * 128],
                                    idf_s[0:K, 0:K])
                            eng = nc.gpsimd if bi % 2 == 0 else nc.vector
                            eng.tensor_copy(STt[:, b0 * K:(b0 + 8) * K], t2[:])
                        stages.append(s)
                    stages.append(lambda: nc.gpsimd.tensor_scalar_mul(
                        STt2[:], STt[:], 2.0))
                    return stages

                stages = resize_stages()
                for c in f2[4:16]:
                    emit_chunk(2, c)
                    for s in stages[:2]:
                        s()
                    stages = stages[2:]
                for s in stages:
                    s()

                # post-resize: drip on (the pre-resize debt drains gradually
                # under ACT's exp backlog), remaining chunks with halves
                # spread WITHIN each pass only: pass acc-lifetimes must stay
                # disjoint along each PSUM-region chain (p2->p1, p0->p3),
                # since a pass's start=True would wipe a still-open previous
                # accumulation in the shared region.
                drip_on[0] = True
                # remaining batches now: their Pool/DVE copies enter the
                # queues ahead of the exp chains
                emit_fab("B", 0)
                emit_fab("B", 8)
                emit_fab("A", 24)
                emit_fab("B", 24)

                def pass_seq(fulls, halves):
                    # halves first-ish: a pass's last acc must be early enough
                    # that the region-chain partner's start never overtakes it
                    stride = max(1, len(fulls) // len(halves))
                    out, fi = [], 0
                    for h in halves:
                        out += [h] + fulls[fi:fi + stride]
                        fi += stride
                    return out + fulls[fi:]

                passmap = {}
                for p, (fs, hs) in ((0, (f0, h0c)), (1, (f1, h1c)),
                                    (2, (f2, h2c)), (3, (f3, h3c))):
                    for c in fs + hs:
                        passmap[id(c)] = p

                def emit_seq(seq):
                    for c in seq:
                        emit_chunk(passmap[id(c)], c)

                emit_seq(pass_seq(f2[16:20], h2c))
                emit_seq(pass_seq(f3, h3c))
                emit_seq(pass_seq(f1, h1c))
                emit_seq(pass_seq(f0, h0c))
                drip(len(debt), force=True)

                # ---- loss tail: per-class sums to host (it sums K classes
                # x 8 cores and applies -WEIGHT/N) ----
                nc.vector.tensor_reduce(
                    pr1[:], partials[:], mybir.AxisListType.X, ALU.add)
                nc.sync.dma_start(out_d[:], pr1[:])

    nc.finalize()
    return nc


_CACHE = {}


def _get_nc():
    if "nc" not in _CACHE:
        _CACHE["nc"] = _build()
    return _CACHE["nc"]


def kernel(images: np.ndarray, segmentations: np.ndarray) -> np.ndarray:
    images = np.ascontiguousarray(np.asarray(images, dtype=np.float32))
    segmentations = np.ascontiguousarray(np.asarray(segmentations, dtype=np.float32))
    assert images.shape == (N, C, H, W) and segmentations.shape == (N, K, 128, 128)
    nc = _get_nc()
    consts = _consts()
    in_maps = [
        {"images": images[n], "segmentations": segmentations[n], **consts}
        for n in range(N)
    ]
    res = run_bass_kernel_spmd(nc, in_maps, list(range(N)))
    total = sum(float(np.asarray(res.results[n]["out"], dtype=np.float64).sum())
                for n in range(N))
    total *= -WEIGHT / N
    return np.array([total], dtype=np.float32)


if __name__ == "__main__":
    rng = np.random.RandomState(0)
    img = rng.rand(N, C, H, W).astype(np.float32) * 255.0
    seg = rng.rand(N, K, 128, 128).astype(np.float32)
    print(kernel(img, seg))
